# revision 13
# baseline (speedup 1.0000x reference)
"""BasisResidualFFN Trainium2 kernel.

Math (per token t):
  recipe_soft = softmax(neuron_recipe, axis=-1)                 [64, 16]
  tr[t, :]    = sum_k w[t,k] * recipe_soft[idx[t,k], :]         [16]
  Y[t, (n,r)] = sum_d x[t,d] * basis_A[n,d,r]
  h[t, r]     = sum_n tr[t,n] * Y[t,(n,r)]
  delta[t, d] = sum_{n,r} basis_A[n,d,r] * tr[t,n] * h[t,r]
  out         = gelu((x + alpha*delta) @ w_up + b_up) @ w_down + b_down

Distribution: pure data parallel. B*S = 4096 tokens sharded 512/core
across 8 NeuronCores; all weights replicated. Everything on device is
computed feature-major (features on partitions, tokens on the free
axis, 512 tokens per matmul) so no on-device activation transposes are
needed anywhere in the FFN; x arrives pre-transposed from the host and
the output is un-transposed on the host. Matmuls run in float32r
(full-rate fp32, tf32-like precision). alpha is folded into the second
copy of basis_A on device (exact, by linearity). DMAs are coalesced
into few large transfers because each dma_start costs ~0.6us of serial
trigger time on the Sync queue.
"""

import numpy as np

import concourse.bass as bass
import concourse.mybir as mybir
import concourse.tile as tile
from concourse import bacc
from concourse.bass import ts
from concourse.bass_utils import run_bass_kernel_spmd

P = 128
NCORES = 8
T = 512            # tokens per core
D = 1024
DFF = 4096
NB = 16            # n_basis
R = 32             # rank
NN = 64            # n_neurons
K = 8              # top-k
DC = D // P        # 8 contraction chunks over d
FT = DFF // P      # 32 ff tiles
DT = D // P        # 8 output d tiles
NRT = (NB * R) // P  # 4 (n,r) tiles
TT = T // P        # 4 token tiles per core

# const blob column layout (f32r blob / f32 blob)
BR_SEL, BR_TREP, BR_QRED, BR_W = 0, 512, 640, 672
BF_ID, BF_IOTA, BF_BU, BF_BD, BF_AL, BF_REC, BF_W = 0, 128, 192, 224, 232, 233, 256

F32 = mybir.dt.float32
F32R = mybir.dt.float32r

_BUILT = [None]


def _build_nc():
    nc = bacc.Bacc(None, target_bir_lowering=False)

    xt_d = nc.dram_tensor("xt", [P, DC, T], F32R, kind="ExternalInput")
    idxw_d = nc.dram_tensor("idxw", [P, TT, 2 * K], F32, kind="ExternalInput")
    blobr_d = nc.dram_tensor("blobr", [P, BR_W], F32R, kind="ExternalInput")
    blobf_d = nc.dram_tensor("blobf", [P, BF_W], F32, kind="ExternalInput")
    a1_d = nc.dram_tensor("a1", [P, DC, NB * R], F32R, kind="ExternalInput")
    a2_d = nc.dram_tensor("a2", [P, NRT, D], F32R, kind="ExternalInput")
    wu_d = nc.dram_tensor("wu", [FT // 2, P, 2, DC, P], F32R, kind="ExternalInput")
    wd_d = nc.dram_tensor("wd", [DT * 2, P, FT // 2, P], F32R, kind="ExternalInput")
    out_d = nc.dram_tensor("outT", [P, DT, T], F32, kind="ExternalOutput")

    AX = mybir.AxisListType.X
    AF = mybir.ActivationFunctionType
    ALU = mybir.AluOpType

    with tile.TileContext(nc) as tc:
        with (
            tc.tile_pool(name="const", bufs=1) as constp,
            tc.tile_pool(name="stream", bufs=3) as stream,
            tc.tile_pool(name="wdstream", bufs=2) as wdstream,
            tc.tile_pool(name="mid", bufs=1) as mid,
            tc.tile_pool(name="small", bufs=2) as small,
            tc.tile_pool(name="psum", bufs=4, space="PSUM") as psum,
            tc.tile_pool(name="psums", bufs=1, space="PSUM") as psums,
        ):
            # ---- resident loads, critical-path first ----
            xt = constp.tile([P, DC, T], F32R, tag="xt")
            nc.sync.dma_start(xt[:], xt_d[:])
            a1 = constp.tile([P, DC, NB * R], F32R, tag="a1")
            nc.sync.dma_start(a1[:], a1_d[:])
            blobf = constp.tile([P, BF_W], F32, tag="blobf")
            nc.sync.dma_start(blobf[:], blobf_d[:])
            blobr = constp.tile([P, BR_W], F32R, tag="blobr")
            nc.sync.dma_start(blobr[:], blobr_d[:])
            idxw = constp.tile([P, TT, 2 * K], F32, tag="idxw")
            nc.sync.dma_start(idxw[:], idxw_d[:])
            a2 = constp.tile([P, NRT, D], F32R, tag="a2")
            nc.sync.dma_start(a2[:], a2_d[:])

            ident = blobf[:, BF_ID:BF_ID + P]
            iota = blobf[:, BF_IOTA:BF_IOTA + NN]
            bu = blobf[:, BF_BU:BF_BU + FT]
            bd = blobf[:, BF_BD:BF_BD + DT]
            alpha = blobf[:, BF_AL:BF_AL + 1]
            rec = blobf[:NN, BF_REC:BF_REC + NB]
            trep = blobr[:R, BR_TREP:BR_TREP + P]
            qred = blobr[:, BR_QRED:BR_QRED + R]

            # fold alpha into A2 (delta path) so x_f = x + deltaT directly
            nc.vector.tensor_scalar_mul(
                a2[:].rearrange("p a b -> p (a b)"),
                a2[:].rearrange("p a b -> p (a b)"), alpha)

            # ---- softmax over the 16-basis axis of the recipe table ----
            mx = small.tile([NN, 1], F32, tag="mx")
            nc.vector.reduce_max(mx[:], rec, axis=AX)
            negmx = small.tile([NN, 1], F32, tag="negmx")
            nc.vector.tensor_scalar_mul(negmx[:], mx[:], -1.0)
            esb = small.tile([NN, NB], F32, tag="esb")
            nc.scalar.activation(esb[:], rec, AF.Exp, bias=negmx[:, 0:1], scale=1.0)
            ssum = small.tile([NN, 1], F32, tag="ssum")
            nc.vector.reduce_sum(ssum[:], esb[:], axis=AX)
            rsum = small.tile([NN, 1], F32, tag="rsum")
            nc.vector.reciprocal(rsum[:], ssum[:])
            recs = constp.tile([NN, NB], F32R, tag="recs")
            nc.vector.tensor_scalar_mul(recs[:], esb[:], rsum[:, 0:1])

            # ---- routing: weighted one-hot scatter S[t, neuron], transposed ----
            st_sb = constp.tile([NN, T], F32R, tag="st")
            for tt in range(TT):
                sk = small.tile([P, NN, K], F32, tag="sk")
                for k in range(K):
                    nc.vector.tensor_scalar(
                        sk[:, :, k], iota,
                        idxw[:, tt, k:k + 1], idxw[:, tt, K + k:K + k + 1],
                        ALU.is_equal, ALU.mult,
                    )
                s_tile = small.tile([P, NN], F32, tag="s")
                nc.vector.reduce_sum(s_tile[:], sk[:], axis=AX)
                stp = psums.tile([NN, P], F32, tag="stp")
                nc.tensor.transpose(stp[:], s_tile[:], ident)
                nc.vector.tensor_copy(st_sb[:, ts(tt, P)], stp[:])

            # token recipes, transposed: recipeT[n, t]
            rt_ps = psums.tile([NB, T], F32, tag="rtps")
            nc.tensor.matmul(rt_ps[:], recs[:], st_sb[:], start=True, stop=True)
            recipeT = constp.tile([NB, T], F32R, tag="recipeT")
            nc.vector.tensor_copy(recipeT[:], rt_ps[:])

            # RepR[(n,r), t] = recipeT[n, t] replicated over r (per nr-tile)
            repr_sb = []
            for i in range(NRT):
                rp = psum.tile([P, T], F32, tag="ps", name=f"rp{i}")
                nc.tensor.matmul(rp[:], blobr[:NB, ts(i, P)], recipeT[:],
                                 start=True, stop=True)
                rr = constp.tile([P, T], F32, tag=f"repr{i}", name=f"repr{i}")
                nc.vector.tensor_copy(rr[:], rp[:])
                repr_sb.append(rr)

            # ---- YT = A1^T @ xT;  WYT = YT * RepR;  hT = sum_n WYT ----
            ht_ps = psums.tile([R, T], F32, tag="htps")
            wyt = [mid.tile([P, T], F32R, tag=f"mid{i}", name=f"wyt{i}")
                   for i in range(NRT)]
            for i in range(NRT):
                yt_ps = psum.tile([P, T], F32, tag="ps", name=f"yt{i}")
                for dc in range(DC):
                    nc.tensor.matmul(yt_ps[:], a1[:, dc, ts(i, P)], xt[:, dc, :],
                                     start=(dc == 0), stop=(dc == DC - 1))
                nc.vector.tensor_mul(out=wyt[i][:], in0=yt_ps[:], in1=repr_sb[i][:])
                nc.tensor.matmul(ht_ps[:], qred, wyt[i][:],
                                 start=(i == 0), stop=(i == NRT - 1))
            ht_sb = constp.tile([R, T], F32R, tag="ht")
            nc.vector.tensor_copy(ht_sb[:], ht_ps[:])

            # ---- CT = RepH * RepR;  deltaT = (alpha*A2)^T @ CT;  xf = x + deltaT ----
            ct = [mid.tile([P, T], F32R, tag=f"mid{i}", name=f"ct{i}")
                  for i in range(NRT)]
            for i in range(NRT):
                rh_ps = psum.tile([P, T], F32, tag="ps", name=f"rh{i}")
                nc.tensor.matmul(rh_ps[:], trep, ht_sb[:], start=True, stop=True)
                nc.vector.tensor_mul(out=ct[i][:], in0=rh_ps[:], in1=repr_sb[i][:])
            # reuses a1's slot: a1's last read (YT matmuls) precedes every xf write
            xf = constp.tile([P, DC, T], F32R, tag="a1", name="xf")
            for dt in range(DT):
                dl_ps = psum.tile([P, T], F32, tag="ps", name=f"dl{dt}")
                for i in range(NRT):
                    nc.tensor.matmul(dl_ps[:], a2[:, i, ts(dt, P)], ct[i][:],
                                     start=(i == 0), stop=(i == NRT - 1))
                nc.vector.tensor_add(out=xf[:, dt, :], in0=dl_ps[:], in1=xt[:, dt, :])

            # ---- FFN up + exact gelu ----
            # reuses xt's slot: xt's last read (xf adds) precedes every g write
            g = constp.tile([P, FT, T], F32R, tag="xt", name="g")
            for ftp in range(FT // 2):
                wu = stream.tile([P, 2, DC, P], F32R, tag="wu", name=f"wu{ftp}")
                nc.sync.dma_start(wu[:], wu_d[ftp])
                for j in range(2):
                    ft = 2 * ftp + j
                    u_ps = psum.tile([P, T], F32, tag="ps", name=f"u{ft}")
                    for dc in range(DC):
                        nc.tensor.matmul(u_ps[:], wu[:, j, dc, :], xf[:, dc, :],
                                         start=(dc == 0), stop=(dc == DC - 1))
                    nc.scalar.activation(g[:, ft, :], u_ps[:], AF.Gelu,
                                         bias=bu[:, ft:ft + 1], scale=1.0)

            # ---- FFN down + bias ----
            for dt in range(DT):
                o_ps = psum.tile([P, T], F32, tag="ps", name=f"o{dt}")
                for h in range(2):
                    wd = wdstream.tile([P, FT // 2, P], F32R, tag="wd",
                                       name=f"wd{dt}_{h}")
                    nc.scalar.dma_start(wd[:], wd_d[dt * 2 + h])
                    for fc in range(FT // 2):
                        fcg = h * (FT // 2) + fc
                        nc.tensor.matmul(o_ps[:], wd[:, fc, :], g[:, fcg, :],
                                         start=(fcg == 0), stop=(fcg == FT - 1))
                ot = stream.tile([P, T], F32, tag="ot", name=f"ot{dt}")
                nc.scalar.activation(ot[:], o_ps[:], AF.Identity,
                                     bias=bd[:, dt:dt + 1], scale=1.0)
                nc.sync.dma_start(out_d[:, dt, :], ot[:])

    nc.finalize()
    return nc


def _get_nc():
    if _BUILT[0] is None:
        _BUILT[0] = _build_nc()
    return _BUILT[0]


def kernel(x, neuron_idx, neuron_weights, neuron_recipe, basis_A,
           w_up_w, w_up_b, w_down_w, w_down_b, alpha):
    nc = _get_nc()

    x = np.asarray(x, dtype=np.float32).reshape(NCORES * T, D)
    idxf = np.asarray(neuron_idx).astype(np.float32).reshape(NCORES * T, K)
    wgt = np.asarray(neuron_weights, dtype=np.float32).reshape(NCORES * T, K)
    rec = np.asarray(neuron_recipe, dtype=np.float32)
    bA = np.asarray(basis_A, dtype=np.float32)
    wu = np.asarray(w_up_w, dtype=np.float32)
    bu_in = np.asarray(w_up_b, dtype=np.float32)
    wd = np.asarray(w_down_w, dtype=np.float32)
    bd_in = np.asarray(w_down_b, dtype=np.float32)
    alpha_f = float(np.asarray(alpha, dtype=np.float32))

    # replicated operands, packed into the on-device layouts
    a1 = np.ascontiguousarray(
        bA.transpose(1, 0, 2).reshape(D, NB * R)
        .reshape(DC, P, NB * R).transpose(1, 0, 2))
    a2 = np.ascontiguousarray(
        bA.transpose(0, 2, 1).reshape(NB * R, D)
        .reshape(NRT, P, D).transpose(1, 0, 2))
    wu_p = np.ascontiguousarray(
        wu.reshape(DC, P, FT // 2, 2, P).transpose(2, 1, 3, 0, 4))
    wd_p = np.ascontiguousarray(
        wd.reshape(2, FT // 2, P, DT, P).transpose(3, 0, 2, 1, 4)
        .reshape(DT * 2, P, FT // 2, P))

    blobf = np.zeros((P, BF_W), dtype=np.float32)
    blobf[:, BF_ID:BF_ID + P] = np.eye(P, dtype=np.float32)
    blobf[:, BF_IOTA:BF_IOTA + NN] = np.arange(NN, dtype=np.float32)[None, :]
    blobf[:, BF_BU:BF_BU + FT] = bu_in.reshape(FT, P).T
    blobf[:, BF_BD:BF_BD + DT] = bd_in.reshape(DT, P).T
    blobf[:, BF_AL] = alpha_f
    blobf[:NN, BF_REC:BF_REC + NB] = rec

    blobr = np.zeros((P, BR_W), dtype=np.float32)
    # SEL[n, i*128+m] = 1 iff n in [4i, 4i+4) and m // 32 == n - 4i
    for n in range(NB):
        i, nloc = divmod(n, NRT)
        blobr[n, BR_SEL + i * P + nloc * R: BR_SEL + i * P + (nloc + 1) * R] = 1.0
    blobr[:R, BR_TREP:BR_TREP + P] = (
        np.arange(P)[None, :] % R == np.arange(R)[:, None])
    blobr[:, BR_QRED:BR_QRED + R] = (
        np.arange(P)[:, None] % R == np.arange(R)[None, :])

    shared = {
        "blobf": blobf, "blobr": blobr, "a1": a1, "a2": a2,
        "wu": wu_p, "wd": wd_p,
    }
    in_maps = []
    idxw = np.concatenate([idxf, wgt], axis=1)  # [N*T, 16]
    for c in range(NCORES):
        xc = x[c * T:(c + 1) * T]  # [T, D]
        xtc = np.ascontiguousarray(xc.T.reshape(DC, P, T).transpose(1, 0, 2))
        iwc = np.ascontiguousarray(
            idxw[c * T:(c + 1) * T].reshape(TT, P, 2 * K).transpose(1, 0, 2))
        in_maps.append({"xt": xtc, "idxw": iwc, **shared})

    res = run_bass_kernel_spmd(nc, in_maps, core_ids=list(range(NCORES)))

    out = np.empty((NCORES * T, D), dtype=np.float32)
    for c in range(NCORES):
        ot = res.results[c]["outT"]  # [P, DT, T]
        out[c * T:(c + 1) * T] = ot.transpose(1, 0, 2).reshape(D, T).T
    return out.reshape(2, 2048, D)


# revision 14
# speedup vs baseline: 1.3149x; 1.3149x over previous
"""BasisResidualFFN Trainium2 kernel.

Math (per token t):
  recipe_soft = softmax(neuron_recipe, axis=-1)                 [64, 16]
  tr[t, :]    = sum_k w[t,k] * recipe_soft[idx[t,k], :]         [16]
  Y[t, (n,r)] = sum_d x[t,d] * basis_A[n,d,r]
  h[t, r]     = sum_n tr[t,n] * Y[t,(n,r)]
  delta[t, d] = sum_{n,r} basis_A[n,d,r] * tr[t,n] * h[t,r]
  out         = gelu((x + alpha*delta) @ w_up + b_up) @ w_down + b_down

Distribution: pure data parallel. B*S = 4096 tokens sharded 512/core
across 8 NeuronCores; all weights replicated. Everything on device is
computed feature-major (features on partitions, tokens on the free
axis, 512 tokens per matmul) so no on-device activation transposes are
needed anywhere in the FFN; x arrives pre-transposed from the host and
the output is un-transposed on the host. Matmuls run in float32r
(full-rate fp32, tf32-like precision). alpha is folded into the second
copy of basis_A on device (exact, by linearity). DMAs are coalesced
into few large transfers because each dma_start costs ~0.6us of serial
trigger time on the Sync queue.
"""

import numpy as np

import concourse.bass as bass
import concourse.mybir as mybir
import concourse.tile as tile
from concourse import bacc
from concourse.bass import ts
from concourse.bass_utils import run_bass_kernel_spmd

P = 128
NCORES = 8
T = 512            # tokens per core
D = 1024
DFF = 4096
NB = 16            # n_basis
R = 32             # rank
NN = 64            # n_neurons
K = 8              # top-k
DC = D // P        # 8 contraction chunks over d
FT = DFF // P      # 32 ff tiles
DT = D // P        # 8 output d tiles
NRT = (NB * R) // P  # 4 (n,r) tiles
TT = T // P        # 4 token tiles per core

# const blob column layout (f32r blob / f32 blob)
BR_SEL, BR_TREP, BR_QRED, BR_W = 0, 512, 640, 672
BF_ID, BF_IOTA, BF_BU, BF_BD, BF_AL, BF_REC, BF_W = 0, 128, 192, 224, 232, 233, 256

F32 = mybir.dt.float32
F32R = mybir.dt.float32r
BF16 = mybir.dt.bfloat16

_BUILT = [None]


def _build_nc():
    nc = bacc.Bacc(None, target_bir_lowering=False)

    xt_d = nc.dram_tensor("xt", [P, DC, T], F32R, kind="ExternalInput")
    idxw_d = nc.dram_tensor("idxw", [P, TT, 2 * K], F32, kind="ExternalInput")
    blobr_d = nc.dram_tensor("blobr", [P, BR_W], F32R, kind="ExternalInput")
    blobf_d = nc.dram_tensor("blobf", [P, BF_W], F32, kind="ExternalInput")
    a1_d = nc.dram_tensor("a1", [P, DC, NB * R], F32R, kind="ExternalInput")
    a2_d = nc.dram_tensor("a2", [P, NRT, D], F32R, kind="ExternalInput")
    wu_d = nc.dram_tensor("wu", [FT // 2, P, 2, DC, P], BF16, kind="ExternalInput")
    wd_d = nc.dram_tensor("wd", [DT * 2, P, FT // 2, P], BF16, kind="ExternalInput")
    out_d = nc.dram_tensor("outT", [P, DT, T], F32, kind="ExternalOutput")

    AX = mybir.AxisListType.X
    AF = mybir.ActivationFunctionType
    ALU = mybir.AluOpType

    with tile.TileContext(nc) as tc:
        with (
            tc.tile_pool(name="const", bufs=1) as constp,
            tc.tile_pool(name="stream", bufs=6) as stream,
            tc.tile_pool(name="wdstream", bufs=4) as wdstream,
            tc.tile_pool(name="mid", bufs=1) as mid,
            tc.tile_pool(name="small", bufs=2) as small,
            tc.tile_pool(name="psum", bufs=4, space="PSUM") as psum,
            tc.tile_pool(name="psums", bufs=1, space="PSUM") as psums,
        ):
            # ---- resident loads: tiny ones first, big ones chunked so the
            # transfers land on parallel HWDGE queues ----
            blobf = constp.tile([P, BF_W], F32, tag="blobf")
            nc.sync.dma_start(blobf[:], blobf_d[:])
            blobr = constp.tile([P, BR_W], F32R, tag="blobr")
            nc.sync.dma_start(blobr[:], blobr_d[:])
            idxw = constp.tile([P, TT, 2 * K], F32, tag="idxw")
            nc.sync.dma_start(idxw[:], idxw_d[:])
            xt = constp.tile([P, DC, T], F32R, tag="xt")
            a1 = constp.tile([P, DC, NB * R], F32R, tag="a1")
            a2 = constp.tile([P, NRT, D], F32R, tag="a2")
            for hh in range(2):
                h4 = ts(hh, DC // 2)
                nc.sync.dma_start(a1[:, h4, :], a1_d[:, h4, :])
                nc.sync.dma_start(xt[:, h4, :], xt_d[:, h4, :])
                nc.sync.dma_start(a2[:, ts(hh, NRT // 2), :], a2_d[:, ts(hh, NRT // 2), :])

            ident = blobf[:, BF_ID:BF_ID + P]
            iota = blobf[:, BF_IOTA:BF_IOTA + NN]
            bu = blobf[:, BF_BU:BF_BU + FT]
            bd = blobf[:, BF_BD:BF_BD + DT]
            alpha = blobf[:, BF_AL:BF_AL + 1]
            rec = blobf[:NN, BF_REC:BF_REC + NB]
            trep = blobr[:R, BR_TREP:BR_TREP + P]
            qred = blobr[:, BR_QRED:BR_QRED + R]

            # fold alpha into A2 (delta path) so x_f = x + deltaT directly
            nc.vector.tensor_scalar_mul(
                a2[:].rearrange("p a b -> p (a b)"),
                a2[:].rearrange("p a b -> p (a b)"), alpha)

            # ---- softmax over the 16-basis axis of the recipe table ----
            mx = small.tile([NN, 1], F32, tag="mx")
            nc.vector.reduce_max(mx[:], rec, axis=AX)
            negmx = small.tile([NN, 1], F32, tag="negmx")
            nc.vector.tensor_scalar_mul(negmx[:], mx[:], -1.0)
            esb = small.tile([NN, NB], F32, tag="esb")
            nc.scalar.activation(esb[:], rec, AF.Exp, bias=negmx[:, 0:1], scale=1.0)
            ssum = small.tile([NN, 1], F32, tag="ssum")
            nc.vector.reduce_sum(ssum[:], esb[:], axis=AX)
            rsum = small.tile([NN, 1], F32, tag="rsum")
            nc.vector.reciprocal(rsum[:], ssum[:])
            recs = constp.tile([NN, NB], F32R, tag="recs")
            nc.vector.tensor_scalar_mul(recs[:], esb[:], rsum[:, 0:1])

            # ---- routing: weighted one-hot scatter S[t, neuron], transposed ----
            st_sb = constp.tile([NN, T], F32R, tag="st")
            for tt in range(TT):
                sk = small.tile([P, NN, K], F32, tag="sk")
                for k in range(K):
                    nc.vector.tensor_scalar(
                        sk[:, :, k], iota,
                        idxw[:, tt, k:k + 1], idxw[:, tt, K + k:K + k + 1],
                        ALU.is_equal, ALU.mult,
                    )
                s_tile = small.tile([P, NN], F32, tag="s")
                nc.vector.reduce_sum(s_tile[:], sk[:], axis=AX)
                stp = psums.tile([NN, P], F32, tag="stp")
                nc.tensor.transpose(stp[:], s_tile[:], ident)
                nc.vector.tensor_copy(st_sb[:, ts(tt, P)], stp[:])

            # token recipes, transposed: recipeT[n, t]
            rt_ps = psums.tile([NB, T], F32, tag="rtps")
            nc.tensor.matmul(rt_ps[:], recs[:], st_sb[:], start=True, stop=True)
            recipeT = constp.tile([NB, T], F32R, tag="recipeT")
            nc.vector.tensor_copy(recipeT[:], rt_ps[:])

            # RepR[(n,r), t] = recipeT[n, t] replicated over r (per nr-tile)
            repr_sb = []
            for i in range(NRT):
                rp = psum.tile([P, T], F32, tag="ps", name=f"rp{i}")
                nc.tensor.matmul(rp[:], blobr[:NB, ts(i, P)], recipeT[:],
                                 start=True, stop=True)
                rr = constp.tile([P, T], F32, tag=f"repr{i}", name=f"repr{i}")
                nc.vector.tensor_copy(rr[:], rp[:])
                repr_sb.append(rr)

            # ---- YT = A1^T @ xT;  WYT = YT * RepR;  hT = sum_n WYT ----
            ht_ps = psums.tile([R, T], F32, tag="htps")
            wyt = [mid.tile([P, T], F32R, tag=f"mid{i}", name=f"wyt{i}")
                   for i in range(NRT)]
            for i in range(NRT):
                yt_ps = psum.tile([P, T], F32, tag="ps", name=f"yt{i}")
                for dc in range(DC):
                    nc.tensor.matmul(yt_ps[:], a1[:, dc, ts(i, P)], xt[:, dc, :],
                                     start=(dc == 0), stop=(dc == DC - 1))
                nc.vector.tensor_mul(out=wyt[i][:], in0=yt_ps[:], in1=repr_sb[i][:])
                nc.tensor.matmul(ht_ps[:], qred, wyt[i][:],
                                 start=(i == 0), stop=(i == NRT - 1))
            ht_sb = constp.tile([R, T], F32R, tag="ht")
            nc.vector.tensor_copy(ht_sb[:], ht_ps[:])

            # ---- CT = RepH * RepR;  deltaT = (alpha*A2)^T @ CT;  xf = x + deltaT ----
            ct = [mid.tile([P, T], F32R, tag=f"mid{i}", name=f"ct{i}")
                  for i in range(NRT)]
            for i in range(NRT):
                rh_ps = psum.tile([P, T], F32, tag="ps", name=f"rh{i}")
                nc.tensor.matmul(rh_ps[:], trep, ht_sb[:], start=True, stop=True)
                nc.vector.tensor_mul(out=ct[i][:], in0=rh_ps[:], in1=repr_sb[i][:])
            # reuses a1's slot: a1's last read (YT matmuls) precedes every xf write
            xf = constp.tile([P, DC, T], BF16, tag="a1", name="xf")
            for dt in range(DT):
                dl_ps = psum.tile([P, T], F32, tag="ps", name=f"dl{dt}")
                for i in range(NRT):
                    nc.tensor.matmul(dl_ps[:], a2[:, i, ts(dt, P)], ct[i][:],
                                     start=(i == 0), stop=(i == NRT - 1))
                nc.vector.tensor_add(out=xf[:, dt, :], in0=dl_ps[:], in1=xt[:, dt, :])

            # ---- FFN up + exact gelu ----
            # reuses xt's slot: xt's last read (xf adds) precedes every g write
            g = constp.tile([P, FT, T], BF16, tag="xt", name="g")
            for ftp in range(FT // 2):
                wu = stream.tile([P, 2, DC, P], BF16, tag="wu", name=f"wu{ftp}")
                nc.sync.dma_start(wu[:], wu_d[ftp])
                for j in range(2):
                    ft = 2 * ftp + j
                    u_ps = psum.tile([P, T], F32, tag="ps", name=f"u{ft}")
                    for dc in range(DC):
                        nc.tensor.matmul(u_ps[:], wu[:, j, dc, :], xf[:, dc, :],
                                         start=(dc == 0), stop=(dc == DC - 1))
                    nc.scalar.activation(g[:, ft, :], u_ps[:], AF.Gelu,
                                         bias=bu[:, ft:ft + 1], scale=1.0)

            # ---- FFN down + bias ----
            for dt in range(DT):
                o_ps = psum.tile([P, T], F32, tag="ps", name=f"o{dt}")
                for h in range(2):
                    wd = wdstream.tile([P, FT // 2, P], BF16, tag="wd",
                                       name=f"wd{dt}_{h}")
                    nc.scalar.dma_start(wd[:], wd_d[dt * 2 + h])
                    for fc in range(FT // 2):
                        fcg = h * (FT // 2) + fc
                        nc.tensor.matmul(o_ps[:], wd[:, fc, :], g[:, fcg, :],
                                         start=(fcg == 0), stop=(fcg == FT - 1))
                ot = stream.tile([P, T], F32, tag="ot", name=f"ot{dt}")
                nc.scalar.activation(ot[:], o_ps[:], AF.Identity,
                                     bias=bd[:, dt:dt + 1], scale=1.0)
                nc.sync.dma_start(out_d[:, dt, :], ot[:])

    nc.finalize()
    return nc


def _get_nc():
    if _BUILT[0] is None:
        _BUILT[0] = _build_nc()
    return _BUILT[0]


def kernel(x, neuron_idx, neuron_weights, neuron_recipe, basis_A,
           w_up_w, w_up_b, w_down_w, w_down_b, alpha):
    nc = _get_nc()

    x = np.asarray(x, dtype=np.float32).reshape(NCORES * T, D)
    idxf = np.asarray(neuron_idx).astype(np.float32).reshape(NCORES * T, K)
    wgt = np.asarray(neuron_weights, dtype=np.float32).reshape(NCORES * T, K)
    rec = np.asarray(neuron_recipe, dtype=np.float32)
    bA = np.asarray(basis_A, dtype=np.float32)
    wu = np.asarray(w_up_w, dtype=np.float32)
    bu_in = np.asarray(w_up_b, dtype=np.float32)
    wd = np.asarray(w_down_w, dtype=np.float32)
    bd_in = np.asarray(w_down_b, dtype=np.float32)
    alpha_f = float(np.asarray(alpha, dtype=np.float32))

    # replicated operands, packed into the on-device layouts
    a1 = np.ascontiguousarray(
        bA.transpose(1, 0, 2).reshape(D, NB * R)
        .reshape(DC, P, NB * R).transpose(1, 0, 2))
    a2 = np.ascontiguousarray(
        bA.transpose(0, 2, 1).reshape(NB * R, D)
        .reshape(NRT, P, D).transpose(1, 0, 2))
    import ml_dtypes
    wu_p = np.ascontiguousarray(
        wu.reshape(DC, P, FT // 2, 2, P).transpose(2, 1, 3, 0, 4)
    ).astype(ml_dtypes.bfloat16)
    wd_p = np.ascontiguousarray(
        wd.reshape(2, FT // 2, P, DT, P).transpose(3, 0, 2, 1, 4)
        .reshape(DT * 2, P, FT // 2, P)).astype(ml_dtypes.bfloat16)

    blobf = np.zeros((P, BF_W), dtype=np.float32)
    blobf[:, BF_ID:BF_ID + P] = np.eye(P, dtype=np.float32)
    blobf[:, BF_IOTA:BF_IOTA + NN] = np.arange(NN, dtype=np.float32)[None, :]
    blobf[:, BF_BU:BF_BU + FT] = bu_in.reshape(FT, P).T
    blobf[:, BF_BD:BF_BD + DT] = bd_in.reshape(DT, P).T
    blobf[:, BF_AL] = alpha_f
    blobf[:NN, BF_REC:BF_REC + NB] = rec

    blobr = np.zeros((P, BR_W), dtype=np.float32)
    # SEL[n, i*128+m] = 1 iff n in [4i, 4i+4) and m // 32 == n - 4i
    for n in range(NB):
        i, nloc = divmod(n, NRT)
        blobr[n, BR_SEL + i * P + nloc * R: BR_SEL + i * P + (nloc + 1) * R] = 1.0
    blobr[:R, BR_TREP:BR_TREP + P] = (
        np.arange(P)[None, :] % R == np.arange(R)[:, None])
    blobr[:, BR_QRED:BR_QRED + R] = (
        np.arange(P)[:, None] % R == np.arange(R)[None, :])

    shared = {
        "blobf": blobf, "blobr": blobr, "a1": a1, "a2": a2,
        "wu": wu_p, "wd": wd_p,
    }
    in_maps = []
    idxw = np.concatenate([idxf, wgt], axis=1)  # [N*T, 16]
    for c in range(NCORES):
        xc = x[c * T:(c + 1) * T]  # [T, D]
        xtc = np.ascontiguousarray(xc.T.reshape(DC, P, T).transpose(1, 0, 2))
        iwc = np.ascontiguousarray(
            idxw[c * T:(c + 1) * T].reshape(TT, P, 2 * K).transpose(1, 0, 2))
        in_maps.append({"xt": xtc, "idxw": iwc, **shared})

    res = run_bass_kernel_spmd(nc, in_maps, core_ids=list(range(NCORES)))

    out = np.empty((NCORES * T, D), dtype=np.float32)
    for c in range(NCORES):
        ot = res.results[c]["outT"]  # [P, DT, T]
        out[c * T:(c + 1) * T] = ot.transpose(1, 0, 2).reshape(D, T).T
    return out.reshape(2, 2048, D)


# revision 16
# speedup vs baseline: 1.3297x; 1.0112x over previous
"""BasisResidualFFN Trainium2 kernel.

Math (per token t):
  recipe_soft = softmax(neuron_recipe, axis=-1)                 [64, 16]
  tr[t, :]    = sum_k w[t,k] * recipe_soft[idx[t,k], :]         [16]
  Y[t, (n,r)] = sum_d x[t,d] * basis_A[n,d,r]
  h[t, r]     = sum_n tr[t,n] * Y[t,(n,r)]
  delta[t, d] = sum_{n,r} basis_A[n,d,r] * tr[t,n] * h[t,r]
  out         = gelu((x + alpha*delta) @ w_up + b_up) @ w_down + b_down

Distribution: pure data parallel. B*S = 4096 tokens sharded 512/core
across 8 NeuronCores; all weights replicated. Everything on device is
computed feature-major (features on partitions, tokens on the free
axis, 512 tokens per matmul) so no on-device activation transposes are
needed anywhere in the FFN; x arrives pre-transposed from the host and
the output is un-transposed on the host.

Precision: the residual path keeps x in float32r (tf32-like matmul
precision at full PE rate). The FFN and the whole basis/routing path
run bf16 — basis-path errors enter the output only through
alpha*delta with alpha ~ 0.1, so their contribution is strongly
damped. DMAs are coalesced into few large transfers because each
dma_start costs ~0.6us of serial trigger time on its queue.
"""

import numpy as np

import concourse.bass as bass
import concourse.mybir as mybir
import concourse.tile as tile
from concourse import bacc
from concourse.bass import ts
from concourse.bass_utils import run_bass_kernel_spmd

P = 128
NCORES = 8
T = 512            # tokens per core
D = 1024
DFF = 4096
NB = 16            # n_basis
R = 32             # rank
NN = 64            # n_neurons
K = 8              # top-k
DC = D // P        # 8 contraction chunks over d
FT = DFF // P      # 32 ff tiles
DT = D // P        # 8 output d tiles
NRT = (NB * R) // P  # 4 (n,r) tiles
TT = T // P        # 4 token tiles per core

# const blob column layouts (bf16 blob / f32 blob)
BR_SEL, BR_TREP, BR_QRED, BR_IOTA, BR_W = 0, 512, 640, 672, 736
BF_ID, BF_BU, BF_BD, BF_AL, BF_REC, BF_W = 0, 128, 160, 168, 169, 192

F32 = mybir.dt.float32
F32R = mybir.dt.float32r
BF16 = mybir.dt.bfloat16

_BUILT = [None]


def _build_nc():
    nc = bacc.Bacc(None, target_bir_lowering=False)

    xt_d = nc.dram_tensor("xt", [P, DC, T], F32R, kind="ExternalInput")
    idxw_d = nc.dram_tensor("idxw", [P, TT, 2 * K], F32, kind="ExternalInput")
    blobr_d = nc.dram_tensor("blobr", [P, BR_W], BF16, kind="ExternalInput")
    blobf_d = nc.dram_tensor("blobf", [P, BF_W], F32, kind="ExternalInput")
    a1_d = nc.dram_tensor("a1", [P, DC, NB * R], BF16, kind="ExternalInput")
    a2_d = nc.dram_tensor("a2", [P, NRT, D], BF16, kind="ExternalInput")
    wu_d = nc.dram_tensor("wu", [FT // 2, P, 2, DC, P], BF16, kind="ExternalInput")
    wd_d = nc.dram_tensor("wd", [DT * 2, P, FT // 2, P], BF16, kind="ExternalInput")
    out_d = nc.dram_tensor("outT", [P, DT, T], F32, kind="ExternalOutput")

    AX = mybir.AxisListType.X
    AF = mybir.ActivationFunctionType
    ALU = mybir.AluOpType

    with tile.TileContext(nc) as tc:
        with (
            tc.tile_pool(name="const", bufs=1) as constp,
            tc.tile_pool(name="stream", bufs=6) as stream,
            tc.tile_pool(name="wdstream", bufs=4) as wdstream,
            tc.tile_pool(name="mid", bufs=1) as mid,
            tc.tile_pool(name="small", bufs=2) as small,
            tc.tile_pool(name="psum", bufs=4, space="PSUM") as psum,
            tc.tile_pool(name="psums", bufs=1, space="PSUM") as psums,
        ):
            # ---- resident loads: tiny ones first, big ones chunked so the
            # transfers land on parallel HWDGE queues ----
            blobr = constp.tile([P, BR_W], BF16, tag="blobr")
            nc.sync.dma_start(blobr[:], blobr_d[:])
            idxw = constp.tile([P, TT, 2 * K], F32, tag="idxw")
            nc.sync.dma_start(idxw[:], idxw_d[:])
            blobf = constp.tile([P, BF_W], F32, tag="blobf")
            nc.sync.dma_start(blobf[:], blobf_d[:])
            a1 = constp.tile([P, DC, NB * R], BF16, tag="a1")
            xt = constp.tile([P, DC, T], F32R, tag="xt")
            a2 = constp.tile([P, NRT, D], BF16, tag="a2")
            nc.sync.dma_start(a1[:], a1_d[:])
            xtb = constp.tile([P, DC, T], BF16, tag="xtb")
            for hh in range(4):
                h2 = ts(hh, DC // 4)
                nc.sync.dma_start(xt[:, h2, :], xt_d[:, h2, :])
                # bf16 shadow of x for the (alpha-damped) basis path
                nc.vector.tensor_copy(xtb[:, h2, :], xt[:, h2, :])
            nc.sync.dma_start(a2[:], a2_d[:])

            ident = blobf[:, BF_ID:BF_ID + P]
            bu = blobf[:, BF_BU:BF_BU + FT]
            bd = blobf[:, BF_BD:BF_BD + DT]
            alpha = blobf[:, BF_AL:BF_AL + 1]
            rec = blobf[:NN, BF_REC:BF_REC + NB]
            trep = blobr[:R, BR_TREP:BR_TREP + P]
            qred = blobr[:, BR_QRED:BR_QRED + R]
            iota = blobr[:, BR_IOTA:BR_IOTA + NN]

            # ---- routing: weighted one-hot scatter S[t, neuron], transposed.
            # Emitted first: this DVE chain is the startup critical path.
            st_sb = constp.tile([NN, T], BF16, tag="st")
            for tt in range(TT):
                sk = small.tile([P, NN, K], BF16, tag="sk")
                for k in range(K):
                    nc.vector.tensor_scalar(
                        sk[:, :, k], iota,
                        idxw[:, tt, k:k + 1], idxw[:, tt, K + k:K + k + 1],
                        ALU.is_equal, ALU.mult,
                    )
                s_tile = small.tile([P, NN], F32, tag="s")
                nc.vector.reduce_sum(s_tile[:], sk[:], axis=AX)
                stp = psums.tile([NN, P], F32, tag="stp")
                nc.tensor.transpose(stp[:], s_tile[:], ident)
                nc.vector.tensor_copy(st_sb[:, ts(tt, P)], stp[:])

            # ---- softmax over the 16-basis axis of the recipe table ----
            mx = small.tile([NN, 1], F32, tag="mx")
            nc.vector.reduce_max(mx[:], rec, axis=AX)
            negmx = small.tile([NN, 1], F32, tag="negmx")
            nc.vector.tensor_scalar_mul(negmx[:], mx[:], -1.0)
            esb = small.tile([NN, NB], F32, tag="esb")
            nc.scalar.activation(esb[:], rec, AF.Exp, bias=negmx[:, 0:1], scale=1.0)
            ssum = small.tile([NN, 1], F32, tag="ssum")
            nc.vector.reduce_sum(ssum[:], esb[:], axis=AX)
            rsum = small.tile([NN, 1], F32, tag="rsum")
            nc.vector.reciprocal(rsum[:], ssum[:])
            recs = constp.tile([NN, NB], BF16, tag="recs")
            nc.vector.tensor_scalar_mul(recs[:], esb[:], rsum[:, 0:1])

            # token recipes, transposed: recipeT[n, t]
            rt_ps = psums.tile([NB, T], F32, tag="rtps")
            nc.tensor.matmul(rt_ps[:], recs[:], st_sb[:], start=True, stop=True)
            recipeT = constp.tile([NB, T], BF16, tag="recipeT")
            nc.vector.tensor_copy(recipeT[:], rt_ps[:])

            # RepR[(n,r), t] = recipeT[n, t] replicated over r (per nr-tile)
            repr_sb = []
            for i in range(NRT):
                rp = psum.tile([P, T], F32, tag="ps", name=f"rp{i}")
                nc.tensor.matmul(rp[:], blobr[:NB, ts(i, P)], recipeT[:],
                                 start=True, stop=True)
                rr = constp.tile([P, T], F32, tag=f"repr{i}", name=f"repr{i}")
                nc.vector.tensor_copy(rr[:], rp[:])
                repr_sb.append(rr)

            # ---- YT = A1^T @ xT;  WYT = YT * RepR;  hT = sum_n WYT ----
            ht_ps = psums.tile([R, T], F32, tag="htps")
            wyt = [mid.tile([P, T], BF16, tag=f"mid{i}", name=f"wyt{i}")
                   for i in range(NRT)]
            for i in range(NRT):
                yt_ps = psum.tile([P, T], F32, tag="ps", name=f"yt{i}")
                for dc in range(DC):
                    nc.tensor.matmul(yt_ps[:], a1[:, dc, ts(i, P)], xtb[:, dc, :],
                                     start=(dc == 0), stop=(dc == DC - 1))
                nc.vector.tensor_mul(out=wyt[i][:], in0=yt_ps[:], in1=repr_sb[i][:])
                nc.tensor.matmul(ht_ps[:], qred, wyt[i][:],
                                 start=(i == 0), stop=(i == NRT - 1))
            ht_sb = constp.tile([R, T], BF16, tag="ht")
            nc.vector.tensor_copy(ht_sb[:], ht_ps[:])

            # fold alpha into A2 (delta path) so x_f = x + deltaT directly
            nc.vector.tensor_scalar_mul(
                a2[:].rearrange("p a b -> p (a b)"),
                a2[:].rearrange("p a b -> p (a b)"), alpha)

            # ---- CT = RepH * RepR;  deltaT = (alpha*A2)^T @ CT;  xf = x + deltaT ----
            rh_ps = psums.tile([P, T], F32, tag="rhps")
            nc.tensor.matmul(rh_ps[:], trep, ht_sb[:], start=True, stop=True)
            ct = [mid.tile([P, T], BF16, tag=f"mid{i}", name=f"ct{i}")
                  for i in range(NRT)]
            for i in range(NRT):
                nc.vector.tensor_mul(out=ct[i][:], in0=rh_ps[:], in1=repr_sb[i][:])
            xf = constp.tile([P, DC, T], BF16, tag="a1", name="xf")
            for dt in range(DT):
                dl_ps = psum.tile([P, T], F32, tag="ps", name=f"dl{dt}")
                for i in range(NRT):
                    nc.tensor.matmul(dl_ps[:], a2[:, i, ts(dt, P)], ct[i][:],
                                     start=(i == 0), stop=(i == NRT - 1))
                nc.vector.tensor_add(out=xf[:, dt, :], in0=dl_ps[:], in1=xt[:, dt, :])

            # ---- FFN up + exact gelu ----
            # g reuses xt's slot: xt's last read (xf adds) precedes every g write
            g = constp.tile([P, FT, T], BF16, tag="xt", name="g")
            for ftp in range(FT // 2):
                wu = stream.tile([P, 2, DC, P], BF16, tag="wu", name=f"wu{ftp}")
                nc.sync.dma_start(wu[:], wu_d[ftp])
                for j in range(2):
                    ft = 2 * ftp + j
                    u_ps = psum.tile([P, T], F32, tag="ps", name=f"u{ft}")
                    for dc in range(DC):
                        nc.tensor.matmul(u_ps[:], wu[:, j, dc, :], xf[:, dc, :],
                                         start=(dc == 0), stop=(dc == DC - 1))
                    nc.scalar.activation(g[:, ft, :], u_ps[:], AF.Gelu,
                                         bias=bu[:, ft:ft + 1], scale=1.0)

            # ---- FFN down + bias ----
            for dt in range(DT):
                o_ps = psum.tile([P, T], F32, tag="ps", name=f"o{dt}")
                for h in range(2):
                    wd = wdstream.tile([P, FT // 2, P], BF16, tag="wd",
                                       name=f"wd{dt}_{h}")
                    nc.scalar.dma_start(wd[:], wd_d[dt * 2 + h])
                    for fc in range(FT // 2):
                        fcg = h * (FT // 2) + fc
                        nc.tensor.matmul(o_ps[:], wd[:, fc, :], g[:, fcg, :],
                                         start=(fcg == 0), stop=(fcg == FT - 1))
                ot = stream.tile([P, T], F32, tag="ot", name=f"ot{dt}")
                nc.scalar.activation(ot[:], o_ps[:], AF.Identity,
                                     bias=bd[:, dt:dt + 1], scale=1.0)
                nc.sync.dma_start(out_d[:, dt, :], ot[:])

    nc.finalize()
    return nc


def _get_nc():
    if _BUILT[0] is None:
        _BUILT[0] = _build_nc()
    return _BUILT[0]


def kernel(x, neuron_idx, neuron_weights, neuron_recipe, basis_A,
           w_up_w, w_up_b, w_down_w, w_down_b, alpha):
    import ml_dtypes
    nc = _get_nc()

    x = np.asarray(x, dtype=np.float32).reshape(NCORES * T, D)
    idxf = np.asarray(neuron_idx).astype(np.float32).reshape(NCORES * T, K)
    wgt = np.asarray(neuron_weights, dtype=np.float32).reshape(NCORES * T, K)
    rec = np.asarray(neuron_recipe, dtype=np.float32)
    bA = np.asarray(basis_A, dtype=np.float32)
    wu = np.asarray(w_up_w, dtype=np.float32)
    bu_in = np.asarray(w_up_b, dtype=np.float32)
    wd = np.asarray(w_down_w, dtype=np.float32)
    bd_in = np.asarray(w_down_b, dtype=np.float32)
    alpha_f = float(np.asarray(alpha, dtype=np.float32))

    # replicated operands, packed into the on-device layouts
    a1 = np.ascontiguousarray(
        bA.transpose(1, 0, 2).reshape(D, NB * R)
        .reshape(DC, P, NB * R).transpose(1, 0, 2)).astype(ml_dtypes.bfloat16)
    a2 = np.ascontiguousarray(
        bA.transpose(0, 2, 1).reshape(NB * R, D)
        .reshape(NRT, P, D).transpose(1, 0, 2)).astype(ml_dtypes.bfloat16)
    wu_p = np.ascontiguousarray(
        wu.reshape(DC, P, FT // 2, 2, P).transpose(2, 1, 3, 0, 4)
    ).astype(ml_dtypes.bfloat16)
    wd_p = np.ascontiguousarray(
        wd.reshape(2, FT // 2, P, DT, P).transpose(3, 0, 2, 1, 4)
        .reshape(DT * 2, P, FT // 2, P)).astype(ml_dtypes.bfloat16)

    blobf = np.zeros((P, BF_W), dtype=np.float32)
    blobf[:, BF_ID:BF_ID + P] = np.eye(P, dtype=np.float32)
    blobf[:, BF_BU:BF_BU + FT] = bu_in.reshape(FT, P).T
    blobf[:, BF_BD:BF_BD + DT] = bd_in.reshape(DT, P).T
    blobf[:, BF_AL] = alpha_f
    blobf[:NN, BF_REC:BF_REC + NB] = rec

    blobr = np.zeros((P, BR_W), dtype=np.float32)
    # SEL[n, i*128+m] = 1 iff n in [4i, 4i+4) and m // 32 == n - 4i
    for n in range(NB):
        i, nloc = divmod(n, NRT)
        blobr[n, BR_SEL + i * P + nloc * R: BR_SEL + i * P + (nloc + 1) * R] = 1.0
    blobr[:R, BR_TREP:BR_TREP + P] = (
        np.arange(P)[None, :] % R == np.arange(R)[:, None])
    blobr[:, BR_QRED:BR_QRED + R] = (
        np.arange(P)[:, None] % R == np.arange(R)[None, :])
    blobr[:, BR_IOTA:BR_IOTA + NN] = np.arange(NN, dtype=np.float32)[None, :]
    blobr = blobr.astype(ml_dtypes.bfloat16)

    shared = {
        "blobf": blobf, "blobr": blobr, "a1": a1, "a2": a2,
        "wu": wu_p, "wd": wd_p,
    }
    in_maps = []
    idxw = np.concatenate([idxf, wgt], axis=1)  # [N*T, 16]
    for c in range(NCORES):
        xc = x[c * T:(c + 1) * T]  # [T, D]
        xtc = np.ascontiguousarray(xc.T.reshape(DC, P, T).transpose(1, 0, 2))
        iwc = np.ascontiguousarray(
            idxw[c * T:(c + 1) * T].reshape(TT, P, 2 * K).transpose(1, 0, 2))
        in_maps.append({"xt": xtc, "idxw": iwc, **shared})

    res = run_bass_kernel_spmd(nc, in_maps, core_ids=list(range(NCORES)))

    out = np.empty((NCORES * T, D), dtype=np.float32)
    for c in range(NCORES):
        ot = res.results[c]["outT"]  # [P, DT, T]
        out[c * T:(c + 1) * T] = ot.transpose(1, 0, 2).reshape(D, T).T
    return out.reshape(2, 2048, D)
